# revision 1
# baseline (speedup 1.0000x reference)
"""GAT (2-layer graph attention network) on 8 Trainium2 NeuronCores.

Strategy (1D node partition, per sharding hint):
  - Each core owns R = N/8 rows (nodes) of the attention matrix.
  - Layer 1: every core computes the full Wh = X @ W1 (cheap, replicated),
    plus score projections s+ = X @ (W1 @ a_first), s- = X @ (W1 @ a_second).
    Scores e[j, i] = leaky_relu(s+_i + s-_j + maskbias) are built in a single
    fused custom DVE op per tile; exp on the scalar engine (batched); the
    masked-softmax numerator AND denominator come out of the aggregation
    matmuls (denominator via a ones-vector matmul on the same z stream).
  - Between layers: AllGather of each core's [R, 64+aux] payload
    (Wh2 = h_local @ W2 plus fused score projections + a ones column).
  - Layer 2: same fused-score pipeline; denominator rides as column 64 of
    the stationary operand (m=65 <= 128), so it is free.

Numerics: matmuls in bf16 (fp32 PSUM accumulate), softmax math exact up to
bf16 rounding; mask handled as additive -100 before leaky_relu: contribution
of masked entries is < 1e-8 relative (exact in effect).
"""

import math
from contextlib import ExitStack
from dataclasses import dataclass

import numpy as np
import ml_dtypes

import concourse.bass as bass
import concourse.mybir as mybir
import concourse.tile as tile
from concourse import bacc
from concourse.bass_utils import run_bass_kernel_spmd

BF16 = ml_dtypes.bfloat16
ALPHA = 0.2
MASKBIAS = -100.0

# --------------------------------------------------------------------------
# Custom fused DVE ops (registered into concourse.dve_ops at import time)
# --------------------------------------------------------------------------

import concourse.dve_ops as dve_ops
from concourse.dve_spec import (
    Spec, Src0, Src1, C0, Zero, lower, maxx, select, _has_src1,
)
from concourse.dve_uop import DveOpSpec


def _make_specs():
    # out = max(y, alpha*y), y = (in0 + s0) + in1
    #   in0 = s1 broadcast [P, R]; s0 = s2 per-partition [P, 1];
    #   in1 = additive mask bias {0, -100}; imm2 = alpha
    from concourse.dve_spec import C2
    _y = (Src0 + C0) + Src1

    def _score_ref(in0, in1, s0, s1, imm2):
        y = in0.astype(np.float32) + s0 + in1.astype(np.float32)
        return np.maximum(y, y * imm2)

    score = Spec(body=maxx(_y, _y * C2), reference=_score_ref)

    # out = in0 > 0 ? in0 : in1 - s0   (elu with in1 = exp(in0), s0 = 1.0)
    def _elu_ref(in0, in1, s0, s1, imm2):
        x = in0.astype(np.float32)
        return np.where(x > 0, x, in1.astype(np.float32) - s0)

    elu = Spec(body=select(Src0 > Zero, Src0, Src1 - C0), reference=_elu_ref)
    return score, elu


def _register(name, spec):
    if name in dve_ops._SUB_OPCODE_FOR_NAME:
        for op in dve_ops.OPS:
            if op.name == name:
                return op
    row = max(dve_ops._SUB_OPCODE_FOR_NAME.values()) + 1
    assert row < 0x20
    shas = {}
    for ver in ("v3", "v4"):
        uops = lower(spec, ver=ver)
        shas[ver] = DveOpSpec(
            name=name, opcode=row, uops=uops, rd1_en=_has_src1(spec)
        ).sha(ver)
    op = dve_ops.DveOp(name, spec, subdim=False, uops_sha=shas)
    dve_ops.OPS.append(op)
    dve_ops.CUSTOM_DVE_SPECS[name] = spec
    dve_ops._SUB_OPCODE_FOR_NAME[name] = row
    return op


_SCORE_SPEC, _ELU_SPEC = _make_specs()
SCORE_LRELU = _register("SCORE_LRELU_GAT", _SCORE_SPEC)
ELU_SEL = _register("ELU_SEL_GAT", _ELU_SPEC)


# --------------------------------------------------------------------------
# Kernel configuration
# --------------------------------------------------------------------------

@dataclass(frozen=True)
class Cfg:
    N: int = 4096      # nodes
    C: int = 512       # input feature dim
    H: int = 128       # hidden per head (must be 128)
    HEADS: int = 4
    F2: int = 64       # output dim
    CORES: int = 8
    GRP: int = 4       # j-tiles per batched exp

    @property
    def R(self): return self.N // self.CORES          # rows per core
    @property
    def JT(self): return self.N // 128                # j tiles
    @property
    def CT(self): return self.C // 128                # input-feature tiles
    @property
    def HH(self): return self.HEADS * self.H          # layer-1 out features
    @property
    def CT2(self): return self.HH // 128              # layer-2 contraction tiles
    @property
    def RT(self): return self.R // 128                # local row tiles
    @property
    def S8(self): return 2 * self.HEADS               # score projections per node
    @property
    def PAY(self): return self.F2 + 4                 # gather payload cols (64|1|s1|s2|pad)
    @property
    def NG(self): return self.JT // self.GRP


FULL = Cfg()


# --------------------------------------------------------------------------
# Device program
# --------------------------------------------------------------------------

def build_gat_nc(cfg: Cfg, collective: bool = True, iters: int = 1,
                 score_plain: bool = False, elu_plain: bool = False,
                 bcast_pe: bool = False, loop_iters: int = 0,
                 phases: str = "full", gather_wh: bool = True,
                 act_lrelu: int = 0):
    dt = mybir.dt.bfloat16
    f32 = mybir.dt.float32
    add = mybir.AluOpType.add
    mult = mybir.AluOpType.mult
    bypass = mybir.AluOpType.bypass
    Exp = mybir.ActivationFunctionType.Exp

    N, C, HEADS, F2, R = cfg.N, cfg.C, cfg.HEADS, cfg.F2, cfg.R
    JT, CT, HH, CT2, RT = cfg.JT, cfg.CT, cfg.HH, cfg.CT2, cfg.RT
    S8, PAY, GRP, NG = cfg.S8, cfg.PAY, cfg.GRP, cfg.NG
    F2p = F2 + 2

    nc = bacc.Bacc(
        "TRN2", target_bir_lowering=False, debug=False, num_devices=cfg.CORES
    )

    # ---- DRAM I/O -------------------------------------------------------
    xt_d = nc.dram_tensor("xt", [128, CT * N], dt, kind="ExternalInput").ap()
    xtl_d = nc.dram_tensor("xtloc", [128, CT * R], dt, kind="ExternalInput").ap()
    mb_d = nc.dram_tensor("mb", [128, JT * R], dt, kind="ExternalInput").ap()
    w1c_d = nc.dram_tensor("w1c", [128, CT * HH], dt, kind="ExternalInput").ap()
    w1t_d = nc.dram_tensor("w1t", [128, HEADS * C], dt, kind="ExternalInput").ap()
    a1p_d = nc.dram_tensor("a1p", [128, HEADS * 2], dt, kind="ExternalInput").ap()
    w2_d = nc.dram_tensor("w2", [128, CT2 * F2p], dt, kind="ExternalInput").ap()
    w2t_d = nc.dram_tensor("w2t", [F2, HH], dt, kind="ExternalInput").ap()
    a2p_d = nc.dram_tensor("a2p", [F2, 2], dt, kind="ExternalInput").ap()
    id_d = nc.dram_tensor("ident", [128, 128], dt, kind="ExternalInput").ap()
    idf_d = nc.dram_tensor("identf", [128, 128], f32, kind="ExternalInput").ap()
    out_d = nc.dram_tensor("out", [R, F2], f32, kind="ExternalOutput").ap()

    with tile.TileContext(nc) as tc, ExitStack() as ctx:
        const = ctx.enter_context(tc.tile_pool(name="const", bufs=1))
        work = ctx.enter_context(tc.tile_pool(name="work", bufs=3))
        wz = ctx.enter_context(tc.tile_pool(name="wz", bufs=3))
        psb = ctx.enter_context(tc.tile_pool(name="psb", bufs=3, space="PSUM"))
        pss = ctx.enter_context(tc.tile_pool(name="pss", bufs=2, space="PSUM"))
        psd = ctx.enter_context(tc.tile_pool(name="psd", bufs=2, space="PSUM"))
        ps2 = ctx.enter_context(tc.tile_pool(name="ps2", bufs=1, space="PSUM"))
        dram = ctx.enter_context(tc.tile_pool(name="dram", bufs=1, space="DRAM"))

        gsend_t = dram.tile([128, RT * PAY], dt)
        if cfg.CORES > 4:
            gfull_t = nc.dram_tensor(
                "gfull_sh", [cfg.CORES * 128, RT * PAY], dt,
                addr_space="Shared").ap()
        else:
            gfull_t = dram.tile([cfg.CORES * 128, RT * PAY], dt)
        whsend_t = dram.tile([128, RT * HH], dt)
        if cfg.CORES > 4:
            whfull_t = nc.dram_tensor(
                "whfull_sh", [cfg.CORES * 128, RT * HH], dt,
                addr_space="Shared").ap()
        else:
            whfull_t = dram.tile([cfg.CORES * 128, RT * HH], dt)

        import contextlib
        loop_cm = (tc.For_i(0, loop_iters, 1) if loop_iters
                   else contextlib.nullcontext())
        with loop_cm:
          for _it in range(iters):
            # ---- constant loads --------------------------------------------
            xt_sb = const.tile([128, CT * N], dt)
            _hx = CT * N // 2
            nc.sync.dma_start(out=xt_sb[:, 0:_hx], in_=xt_d[:, 0:_hx])
            nc.sync.dma_start(out=xt_sb[:, _hx:], in_=xt_d[:, _hx:])
            xtl_sb = const.tile([128, CT * R], dt)
            nc.gpsimd.dma_start(out=xtl_sb, in_=xtl_d)
            mb_sb = const.tile([128, JT * R], dt)
            _hm = JT * R // 4
            for _q in range(4):
                nc.gpsimd.dma_start(
                    out=mb_sb[:, _q * _hm: (_q + 1) * _hm],
                    in_=mb_d[:, _q * _hm: (_q + 1) * _hm])
            w1c_sb = const.tile([128, CT * HH], dt)
            nc.sync.dma_start(out=w1c_sb, in_=w1c_d)
            w1t_sb = const.tile([128, HEADS * C], dt)
            nc.sync.dma_start(out=w1t_sb, in_=w1t_d)
            a1p_sb = const.tile([128, HEADS * 2], dt)
            nc.sync.dma_start(out=a1p_sb, in_=a1p_d)
            w2a_sb = const.tile([128, CT2 * F2p], dt)
            nc.sync.dma_start(out=w2a_sb, in_=w2_d)
            w2t_sb = const.tile([F2, HH], dt)
            nc.sync.dma_start(out=w2t_sb, in_=w2t_d)
            a2p_sb = const.tile([F2, 2], dt)
            nc.sync.dma_start(out=a2p_sb, in_=a2p_d)
            ident_sb = const.tile([128, 128], dt)
            nc.sync.dma_start(out=ident_sb, in_=id_d)
            identf_sb = const.tile([128, 128], f32)
            nc.sync.dma_start(out=identf_sb, in_=idf_d)

            ones_col = const.tile([128, 1], dt)
            nc.vector.memset(ones_col, 1.0)
            ones_row = const.tile([1, 128], dt)
            nc.vector.memset(ones_row, 1.0)
            onesf_row = const.tile([1, 128], f32)
            nc.vector.memset(onesf_row, 1.0)

            # ---- fused score-projection weights: w~ = W @ a_half -----------
            ws1_sb = const.tile([128, CT * S8], dt)  # [c%128, ct*S8 + 2h+half]
            for h in range(HEADS):
                for ct in range(CT):
                    pw = pss.tile([128, 2], f32, tag="sm")
                    nc.tensor.matmul(
                        out=pw,
                        lhsT=w1t_sb[:, h * C + ct * 128: h * C + (ct + 1) * 128],
                        rhs=a1p_sb[:, h * 2: h * 2 + 2],
                        start=True, stop=True,
                    )
                    nc.vector.tensor_copy(
                        out=ws1_sb[:, ct * S8 + 2 * h: ct * S8 + 2 * h + 2], in_=pw
                    )
            for ct in range(CT2):
                pw = pss.tile([128, 2], f32, tag="sm")
                nc.tensor.matmul(
                    out=pw,
                    lhsT=w2t_sb[0:F2, ct * 128: (ct + 1) * 128],
                    rhs=a2p_sb[0:F2, :],
                    start=True, stop=True,
                )
                nc.vector.tensor_copy(
                    out=w2a_sb[:, ct * F2p + F2: ct * F2p + F2 + 2], in_=pw
                )

            if phases == "dma":
                for rt in range(RT):
                    nc.sync.dma_start(out=out_d[rt * 128:(rt + 1) * 128, :],
                                      in_=identf_sb[:, 0:F2])
                continue
            # ---- layer-1 Wh (all heads) + per-node score projections -------
            wh_sb = const.tile([128, JT * HH], dt)   # Wh[j, :] tiles
            ssb = const.tile([128, JT * S8], f32)    # s projections per j
            for t in range(JT):
                pS = pss.tile([128, S8], f32, tag="sm")
                for ct in range(CT):
                    xsl = xt_sb[:, ct * N + t * 128: ct * N + (t + 1) * 128]
                    nc.tensor.matmul(
                        out=pS, lhsT=xsl, rhs=ws1_sb[:, ct * S8: (ct + 1) * S8],
                        start=(ct == 0), stop=(ct == CT - 1),
                    )
                nc.vector.tensor_copy(out=ssb[:, t * S8: (t + 1) * S8], in_=pS)
                if not gather_wh:
                    pA = psb.tile([128, HH], f32, tag="big")
                    for ct in range(CT):
                        xsl = xt_sb[:, ct * N + t * 128: ct * N + (t + 1) * 128]
                        nc.tensor.matmul(
                            out=pA, lhsT=xsl,
                            rhs=w1c_sb[:, ct * HH: (ct + 1) * HH],
                            start=(ct == 0), stop=(ct == CT - 1),
                        )
                    eng = nc.vector if (t % 2 == 0) else nc.scalar
                    if eng is nc.vector:
                        eng.tensor_copy(out=wh_sb[:, t * HH: (t + 1) * HH], in_=pA)
                    else:
                        eng.copy(out=wh_sb[:, t * HH: (t + 1) * HH], in_=pA)

            if gather_wh:
                # local Wh rows + AllGather, instead of replicated compute
                for rt in range(RT):
                    pA = psb.tile([128, HH], f32, tag="big")
                    for ct in range(CT):
                        nc.tensor.matmul(
                            out=pA,
                            lhsT=xtl_sb[:, ct * R + rt * 128: ct * R + (rt + 1) * 128],
                            rhs=w1c_sb[:, ct * HH: (ct + 1) * HH],
                            start=(ct == 0), stop=(ct == CT - 1),
                        )
                    whl = work.tile([128, HH], dt, tag="whl")
                    nc.vector.tensor_copy(out=whl, in_=pA)
                    nc.sync.dma_start(
                        out=whsend_t[:, rt * HH: (rt + 1) * HH], in_=whl)
                if collective:
                    nc.gpsimd.collective_compute(
                        "AllGather", bypass,
                        replica_groups=[list(range(cfg.CORES))],
                        ins=[whsend_t.opt()], outs=[whfull_t.opt()],
                    )
                else:
                    for c in range(cfg.CORES):
                        nc.sync.dma_start(
                            out=whfull_t[c * 128: (c + 1) * 128, :],
                            in_=whsend_t[:, :])
                for c in range(cfg.CORES):
                    nc.sync.dma_start(
                        out=wh_sb[:, c * RT * HH: (c + 1) * RT * HH],
                        in_=whfull_t[c * 128: (c + 1) * 128, :])
            if phases == "wh":
                for rt in range(RT):
                    nc.sync.dma_start(out=out_d[rt * 128:(rt + 1) * 128, :],
                                      in_=identf_sb[:, 0:F2])
                continue
            # ---- layer 1: per-head attention + aggregation ------------------
            hloc_sb = const.tile([128, CT2 * R], dt)  # h_local^T, feature-major
            s1r_sb = const.tile([1, HEADS * R], dt)
            for h in range(HEADS):
                ps1 = psd.tile([1, R], f32, tag="den")
                for ct in range(CT):
                    nc.tensor.matmul(
                        out=ps1,
                        lhsT=ws1_sb[:, ct * S8 + 2 * h: ct * S8 + 2 * h + 1],
                        rhs=xtl_sb[:, ct * R: (ct + 1) * R],
                        start=(ct == 0), stop=(ct == CT - 1),
                    )
                nc.vector.tensor_copy(out=s1r_sb[0:1, h * R: (h + 1) * R], in_=ps1)
                s1b = work.tile([128, R], dt, tag="s1b")
                if bcast_pe:
                    psb1 = psb.tile([128, R], f32, tag="big")
                    nc.tensor.matmul(out=psb1, lhsT=ones_row,
                                     rhs=s1r_sb[0:1, h * R: (h + 1) * R],
                                     start=True, stop=True)
                    nc.scalar.copy(out=s1b[:, :], in_=psb1)
                else:
                    nc.gpsimd.partition_broadcast(
                        out_ap=s1b[:, :], in_ap=s1r_sb[0:1, h * R: (h + 1) * R]
                    )

                psum_h = psb.tile([128, R], f32, tag="big")
                psum_d = psd.tile([1, R], f32, tag="den")
                for g in range(NG):
                    ug = wz.tile([128, GRP * R], dt, tag="ug")
                    zg = wz.tile([128, GRP * R], dt, tag="zg")
                    for k in range(GRP):
                        t = g * GRP + k
                        if score_plain:
                            nc.vector.tensor_tensor(
                                out=ug[:, k * R: (k + 1) * R], in0=s1b[:, :],
                                in1=mb_sb[:, t * R: (t + 1) * R], op=add)
                        elif k < act_lrelu:
                            u0 = work.tile([128, R], dt, tag="u0")
                            nc.vector.tensor_tensor(
                                out=u0, in0=s1b[:, :],
                                in1=mb_sb[:, t * R: (t + 1) * R], op=add)
                            nc.scalar.activation(
                                out=ug[:, k * R: (k + 1) * R], in_=u0,
                                func=mybir.ActivationFunctionType.Lrelu,
                                bias=ssb[:, t * S8 + 2 * h + 1: t * S8 + 2 * h + 2],
                                scale=1.0, alpha=ALPHA,
                            )
                        else:
                            nc.vector._custom_dve(
                                SCORE_LRELU,
                                out=ug[:, k * R: (k + 1) * R],
                                in0=s1b[:, :],
                                in1=mb_sb[:, t * R: (t + 1) * R],
                                s0=ssb[:, t * S8 + 2 * h + 1: t * S8 + 2 * h + 2],
                                s1=0.0,
                                imm2=ALPHA,
                            )
                    nc.scalar.activation(out=zg[:, :], in_=ug[:, :], func=Exp)
                    for k in range(GRP):
                        t = g * GRP + k
                        zt = zg[:, k * R: (k + 1) * R]
                        nc.tensor.matmul(
                            out=psum_h,
                            lhsT=wh_sb[:, t * HH + h * 128: t * HH + (h + 1) * 128],
                            rhs=zt,
                            start=(t == 0), stop=(t == JT - 1),
                        )
                        nc.tensor.matmul(
                            out=psum_d, lhsT=ones_col, rhs=zt,
                            start=(t == 0), stop=(t == JT - 1),
                        )
                # normalize + elu -> h_local^T tile for this head
                rcp = work.tile([1, R], f32, tag="rcp")
                nc.vector.reciprocal(out=rcp, in_=psum_d[0:1, :])
                prb = psb.tile([128, R], f32, tag="big")
                nc.tensor.matmul(
                    out=prb, lhsT=onesf_row, rhs=rcp[0:1, :], start=True, stop=True
                )
                rb_sb = work.tile([128, R], f32, tag="rb")
                nc.scalar.copy(out=rb_sb, in_=prb)
                hn = work.tile([128, R], f32, tag="hn")
                nc.vector.tensor_tensor(out=hn, in0=psum_h, in1=rb_sb, op=mult)
                eh = work.tile([128, R], dt, tag="eh")
                nc.scalar.activation(out=eh, in_=hn, func=Exp)
                if elu_plain:
                    nc.vector.tensor_copy(
                        out=hloc_sb[:, h * R: (h + 1) * R], in_=hn)
                else:
                    nc.vector._custom_dve(
                        ELU_SEL,
                        out=hloc_sb[:, h * R: (h + 1) * R],
                        in0=hn, in1=eh, s0=1.0, s1=0.0, imm2=0.0,
                    )

            if phases == "l1":
                for rt in range(RT):
                    nc.sync.dma_start(out=out_d[rt * 128:(rt + 1) * 128, :],
                                      in_=identf_sb[:, 0:F2])
                continue
            # ---- layer-2 local projections + gather payload -----------------
            gs_sb = const.tile([128, RT * PAY], dt)
            for rt in range(RT):
                pW = pss.tile([128, F2p], f32, tag="sm")
                for ct in range(CT2):
                    nc.tensor.matmul(
                        out=pW,
                        lhsT=hloc_sb[:, ct * R + rt * 128: ct * R + (rt + 1) * 128],
                        rhs=w2a_sb[:, ct * F2p: (ct + 1) * F2p],
                        start=(ct == 0), stop=(ct == CT2 - 1),
                    )
                b = rt * PAY
                nc.vector.tensor_copy(out=gs_sb[:, b: b + F2], in_=pW[:, 0:F2])
                nc.vector.memset(gs_sb[:, b + F2: b + F2 + 1], 1.0)
                nc.vector.tensor_copy(
                    out=gs_sb[:, b + F2 + 1: b + F2 + 3], in_=pW[:, F2: F2 + 2]
                )
                nc.vector.memset(gs_sb[:, b + F2 + 3: b + PAY], 0.0)
                nc.sync.dma_start(
                    out=gsend_t[:, b: b + PAY], in_=gs_sb[:, b: b + PAY],
                )

            if collective:
                nc.gpsimd.collective_compute(
                    "AllGather",
                    bypass,
                    replica_groups=[list(range(cfg.CORES))],
                    ins=[gsend_t.opt()],
                    outs=[gfull_t.opt()],
                )
            else:
                # timing-only variant (TimelineSim can't model collectives):
                # approximate the gather with DMAs of the same total volume
                for c in range(cfg.CORES):
                    nc.sync.dma_start(
                        out=gfull_t[c * 128: (c + 1) * 128, :],
                        in_=gsend_t[:, :]
                    )

            # ---- layer-2 prep ----------------------------------------------
            gf_sb = const.tile([128, JT * PAY], dt)
            for c in range(cfg.CORES):
                nc.sync.dma_start(
                    out=gf_sb[:, c * RT * PAY: (c + 1) * RT * PAY],
                    in_=gfull_t[c * 128: (c + 1) * 128, :],
                )
            s2pf = const.tile([128, JT], f32)
            nc.vector.tensor_copy(
                out=s2pf[:, :].rearrange("p (t o) -> p t o", o=1),
                in_=gf_sb[:, :].rearrange("p (t q) -> p t q", q=PAY)[
                    :, :, F2 + 2: F2 + 3
                ],
            )
            s1r2_sb = const.tile([1, R], dt)
            for rt in range(RT):
                pt12 = pss.tile([2, 128], dt, tag="sm")
                nc.tensor.transpose(
                    out=pt12,
                    in_=gs_sb[:, rt * PAY + F2 + 1: rt * PAY + F2 + 3],
                    identity=ident_sb,
                )
                nc.vector.tensor_copy(
                    out=s1r2_sb[0:1, rt * 128: (rt + 1) * 128], in_=pt12[0:1, :]
                )
            s1b2 = const.tile([128, R], dt)
            if bcast_pe:
                psb2 = psb.tile([128, R], f32, tag="big")
                nc.tensor.matmul(out=psb2, lhsT=ones_row, rhs=s1r2_sb[0:1, :],
                                 start=True, stop=True)
                nc.scalar.copy(out=s1b2[:, :], in_=psb2)
            else:
                nc.gpsimd.partition_broadcast(out_ap=s1b2[:, :], in_ap=s1r2_sb[0:1, :])

            # ---- layer-2 attention + aggregation ----------------------------
            psum2 = ps2.tile([F2 + 1, R], f32)
            for g in range(NG):
                ug = wz.tile([128, GRP * R], dt, tag="ug")
                zg = wz.tile([128, GRP * R], dt, tag="zg")
                for k in range(GRP):
                    t = g * GRP + k
                    if score_plain:
                        nc.vector.tensor_tensor(
                            out=ug[:, k * R: (k + 1) * R], in0=s1b2[:, :],
                            in1=mb_sb[:, t * R: (t + 1) * R], op=add)
                    elif k < act_lrelu:
                        u0 = work.tile([128, R], dt, tag="u0")
                        nc.vector.tensor_tensor(
                            out=u0, in0=s1b2[:, :],
                            in1=mb_sb[:, t * R: (t + 1) * R], op=add)
                        nc.scalar.activation(
                            out=ug[:, k * R: (k + 1) * R], in_=u0,
                            func=mybir.ActivationFunctionType.Lrelu,
                            bias=s2pf[:, t: t + 1],
                            scale=1.0, alpha=ALPHA,
                        )
                    else:
                        nc.vector._custom_dve(
                            SCORE_LRELU,
                            out=ug[:, k * R: (k + 1) * R],
                            in0=s1b2[:, :],
                            in1=mb_sb[:, t * R: (t + 1) * R],
                            s0=s2pf[:, t: t + 1],
                            s1=0.0,
                            imm2=ALPHA,
                        )
                nc.scalar.activation(out=zg[:, :], in_=ug[:, :], func=Exp)
                for k in range(GRP):
                    t = g * GRP + k
                    nc.tensor.matmul(
                        out=psum2,
                        lhsT=gf_sb[:, t * PAY: t * PAY + F2 + 1],
                        rhs=zg[:, k * R: (k + 1) * R],
                        start=(t == 0), stop=(t == JT - 1),
                    )

            # ---- finalize: transpose, normalize, store ----------------------
            o2 = const.tile([F2 + 1, R], f32)
            nc.vector.tensor_copy(out=o2, in_=psum2)
            for rt in range(RT):
                pT2 = pss.tile([128, F2 + 1], f32, tag="sm")
                nc.tensor.transpose(
                    out=pT2,
                    in_=o2[:, rt * 128: (rt + 1) * 128],
                    identity=identf_sb[0: F2 + 1, 0: F2 + 1],
                )
                rc = work.tile([128, 1], f32, tag="rc")
                nc.vector.reciprocal(out=rc, in_=pT2[:, F2: F2 + 1])
                of = work.tile([128, F2], f32, tag="of")
                nc.vector.tensor_scalar(
                    out=of, in0=pT2[:, 0:F2], scalar1=rc, scalar2=0.0,
                    op0=mult, op1=bypass,
                )
                nc.sync.dma_start(
                    out=out_d[rt * 128: (rt + 1) * 128, :], in_=of
                )

    nc.compile()
    return nc


# --------------------------------------------------------------------------
# Host-side prep / sharding
# --------------------------------------------------------------------------

def host_prep(cfg: Cfg, g, inputs, W1, a1, W2, a2):
    N, C, H, HEADS, F2, R = cfg.N, cfg.C, cfg.H, cfg.HEADS, cfg.F2, cfg.R
    X = np.asarray(inputs, np.float32)
    W1 = np.asarray(W1, np.float32)
    a1 = np.asarray(a1, np.float32)
    W2 = np.asarray(W2, np.float32)
    a2 = np.asarray(a2, np.float32)

    def tile128(A):
        # [k*128, cols] row-major -> partition-major [128, k*cols]
        k = A.shape[0] // 128
        return np.ascontiguousarray(
            A.reshape(k, 128, A.shape[1]).transpose(1, 0, 2).reshape(128, -1)
        )

    XT = np.ascontiguousarray(X.T).astype(BF16)                       # [C, N]
    xt_t = tile128(XT)
    w1c = tile128(np.ascontiguousarray(
        W1.transpose(1, 0, 2).reshape(C, HEADS * H)).astype(BF16))
    w1t = tile128(np.ascontiguousarray(
        W1.transpose(0, 2, 1).reshape(HEADS * H, C)).astype(BF16))
    a1p = tile128(np.ascontiguousarray(
        np.stack([a1[:, :H, 0], a1[:, H:, 0]], axis=-1).reshape(HEADS * H, 2)
    ).astype(BF16))
    CT2 = (HEADS * H) // 128
    F2p = F2 + 2
    w2_tiled = tile128(np.ascontiguousarray(W2).astype(BF16))         # [128, CT2*F2]
    w2a = np.zeros((128, CT2 * F2p), BF16)
    for ct in range(CT2):
        w2a[:, ct * F2p: ct * F2p + F2] = w2_tiled[:, ct * F2: (ct + 1) * F2]
    w2t = np.ascontiguousarray(W2.T).astype(BF16)                     # [F2, HH]
    a2p = np.ascontiguousarray(
        np.stack([a2[:F2, 0], a2[F2:, 0]], axis=-1)
    ).astype(BF16)                                                    # [F2, 2]
    ident = np.eye(128, dtype=BF16)
    identf = np.eye(128, dtype=np.float32)

    adj = np.asarray(g) > 0
    in_maps = []
    for c in range(cfg.CORES):
        rows = slice(c * R, (c + 1) * R)
        mb = np.where(adj[rows].T, 0.0, MASKBIAS).astype(BF16)        # [N, R]
        in_maps.append({
            "xt": xt_t, "xtloc": tile128(np.ascontiguousarray(XT[:, rows])),
            "mb": tile128(np.ascontiguousarray(mb)),
            "w1c": w1c, "w1t": w1t, "a1p": a1p,
            "w2": w2a, "w2t": w2t, "a2p": a2p,
            "ident": ident, "identf": identf,
        })
    return in_maps


_NC_CACHE = {}


def get_compiled(cfg: Cfg):
    nc = _NC_CACHE.get(cfg)
    if nc is None:
        nc = build_gat_nc(cfg)
        _NC_CACHE[cfg] = nc
    return nc


def kernel(g, inputs, W1, a1, W2, a2):
    cfg = FULL
    nc = get_compiled(cfg)
    in_maps = host_prep(cfg, g, inputs, W1, a1, W2, a2)
    res = run_bass_kernel_spmd(nc, in_maps, core_ids=list(range(cfg.CORES)))
    out = np.concatenate(
        [np.asarray(res.results[c]["out"], np.float32) for c in range(cfg.CORES)],
        axis=0,
    )
    return out



# revision 40
# speedup vs baseline: 1.2909x; 1.2909x over previous
"""GAT (2-layer graph attention network) on 8 Trainium2 NeuronCores.

Strategy (1D node partition):
  - Each core owns R = N/8 rows (nodes) of the attention matrix.
  - Exp-free scores: softmax over column i is invariant to any per-i scale,
    so e^{lrelu(s1_i+s2_j)} / e^{s1_i} = max(q_j, w_i*u_j) with per-node
    q = e^{s2}, u = e^{0.2 s2}, w = e^{-0.8 s1}.  The N^2 path is then a
    single 4x-mode tensor_scalar (mult+max) plus one tensor_tensor min with
    a {0,K} mask tile (masking via min is exact: max(q,wu) > 0).  No
    activation-engine pass over N^2 elements at all.
  - Aggregation is "flipped": z-blocks [j,128i] are the PE stationary and
    [Wh_h | ones] streams 129 columns -> out[i, 129] accumulated over j in
    PSUM; column 128 is the softmax denominator for free (no separate
    ones-vector matmuls).
  - AllGather payload per node: 4x(Wh_h|1) + q,u per head (pre-exponentiated
    locally) = 524 cols; no replicated X compute, no full-N projections.
  - Layer 2 runs the same scheme with payload [Wh2|1|q2|u2].
"""

import math
from contextlib import ExitStack
from dataclasses import dataclass

import numpy as np
import ml_dtypes

import concourse.bass as bass
import concourse.mybir as mybir
import concourse.tile as tile
from concourse import bacc
from concourse.bass_utils import run_bass_kernel_spmd

BF16 = ml_dtypes.bfloat16
ALPHA = 0.2
MASK_K = 1.0

# --------------------------------------------------------------------------
# Custom fused DVE op (elu select), registered into concourse.dve_ops
# --------------------------------------------------------------------------

import concourse.dve_ops as dve_ops
from concourse.dve_spec import (
    Spec, Src0, Src1, C0, Zero, lower, select, _has_src1,
)
from concourse.dve_uop import DveOpSpec


def _make_elu_spec():
    # out = in0 > 0 ? in0 : in1 - s0   (elu with in1 = exp(in0), s0 = 1.0)
    def _elu_ref(in0, in1, s0, s1, imm2):
        x = in0.astype(np.float32)
        return np.where(x > 0, x, in1.astype(np.float32) - s0)

    return Spec(body=select(Src0 > Zero, Src0, Src1 - C0), reference=_elu_ref)


def _register(name, spec):
    if name in dve_ops._SUB_OPCODE_FOR_NAME:
        for op in dve_ops.OPS:
            if op.name == name:
                return op
    row = max(dve_ops._SUB_OPCODE_FOR_NAME.values()) + 1
    assert row < 0x20
    shas = {}
    for ver in ("v3", "v4"):
        uops = lower(spec, ver=ver)
        shas[ver] = DveOpSpec(
            name=name, opcode=row, uops=uops, rd1_en=_has_src1(spec)
        ).sha(ver)
    op = dve_ops.DveOp(name, spec, subdim=False, uops_sha=shas)
    dve_ops.OPS.append(op)
    dve_ops.CUSTOM_DVE_SPECS[name] = spec
    dve_ops._SUB_OPCODE_FOR_NAME[name] = row
    return op


ELU_SEL = _register("ELU_SEL_GAT", _make_elu_spec())


# --------------------------------------------------------------------------
# Kernel configuration
# --------------------------------------------------------------------------

@dataclass(frozen=True)
class Cfg:
    N: int = 4096      # nodes
    C: int = 512       # input feature dim
    H: int = 128       # hidden per head (must be 128)
    HEADS: int = 4
    F2: int = 64       # output dim
    CORES: int = 8
    GRP: int = 8       # j-tiles per batched mask-min

    @property
    def R(self): return self.N // self.CORES          # rows per core
    @property
    def JT(self): return self.N // 128                # j tiles
    @property
    def CT(self): return self.C // 128                # input-feature tiles
    @property
    def HH(self): return self.HEADS * self.H          # layer-1 out features
    @property
    def CT2(self): return self.HH // 128              # layer-2 contraction tiles
    @property
    def RT(self): return self.R // 128                # local row tiles
    @property
    def PW1(self): return self.HEADS * 129 + 2 * self.HEADS  # L1 payload cols
    @property
    def PAY2(self): return self.F2 + 3                # L2 payload cols
    @property
    def NG(self): return self.JT // self.GRP


FULL = Cfg()


# --------------------------------------------------------------------------
# Device program
# --------------------------------------------------------------------------

def build_gat_nc(cfg: Cfg, collective: bool = True, iters: int = 1,
                 loop_iters: int = 0, phases: str = "full",
                 pool_den: int = 8, pool_num: int = 3, dump: str = ""):
    dt = mybir.dt.bfloat16
    f32 = mybir.dt.float32
    add = mybir.AluOpType.add
    mult = mybir.AluOpType.mult
    mx = mybir.AluOpType.max
    mn = mybir.AluOpType.min
    bypass = mybir.AluOpType.bypass
    Exp = mybir.ActivationFunctionType.Exp
    Copy = mybir.ActivationFunctionType.Copy

    N, C, HEADS, F2, R = cfg.N, cfg.C, cfg.HEADS, cfg.F2, cfg.R
    JT, CT, HH, CT2, RT = cfg.JT, cfg.CT, cfg.HH, cfg.CT2, cfg.RT
    GRP, NG, PW1, PAY2 = cfg.GRP, cfg.NG, cfg.PW1, cfg.PAY2
    F2p = F2 + 2
    QOFF = HEADS * 129            # offset of q cols within a payload block
    UOFF = QOFF + HEADS           # offset of u cols

    nc = bacc.Bacc(
        "TRN2", target_bir_lowering=False, debug=False, num_devices=cfg.CORES
    )

    # ---- DRAM I/O -------------------------------------------------------
    xtl_d = nc.dram_tensor("xtloc", [128, CT * R], dt, kind="ExternalInput").ap()
    km_d = nc.dram_tensor("km", [128, JT * R], dt, kind="ExternalInput").ap()
    w1c_d = nc.dram_tensor("w1c", [128, CT * HH], dt, kind="ExternalInput").ap()
    ws1_d = nc.dram_tensor("ws1", [128, CT * 8], dt, kind="ExternalInput").ap()
    w2a_d = nc.dram_tensor("w2a", [128, CT2 * F2p], dt, kind="ExternalInput").ap()
    id_d = nc.dram_tensor("ident", [128, 128], dt, kind="ExternalInput").ap()
    out_d = nc.dram_tensor("out", [R, F2], f32, kind="ExternalOutput").ap()

    with tile.TileContext(nc) as tc, ExitStack() as ctx:
        const = ctx.enter_context(tc.tile_pool(name="const", bufs=1))
        work = ctx.enter_context(tc.tile_pool(name="work", bufs=3))
        wz = ctx.enter_context(tc.tile_pool(name="wz", bufs=3))
        psn = ctx.enter_context(tc.tile_pool(name="psn", bufs=1, space="PSUM"))
        pstp = ctx.enter_context(tc.tile_pool(name="pstp", bufs=2, space="PSUM"))
        dram = ctx.enter_context(tc.tile_pool(name="dram", bufs=1, space="DRAM"))

        whsend_t = dram.tile([128, RT * PW1], dt)
        gsend_t = dram.tile([128, RT * PAY2], dt)
        if cfg.CORES > 4:
            whfull_t = nc.dram_tensor(
                "whfull_sh", [cfg.CORES * 128, RT * PW1], dt,
                addr_space="Shared").ap()
            gfull_t = nc.dram_tensor(
                "gfull_sh", [cfg.CORES * 128, RT * PAY2], dt,
                addr_space="Shared").ap()
        else:
            whfull_t = dram.tile([cfg.CORES * 128, RT * PW1], dt)
            gfull_t = dram.tile([cfg.CORES * 128, RT * PAY2], dt)

        import contextlib
        loop_cm = (tc.For_i(0, loop_iters, 1) if loop_iters
                   else contextlib.nullcontext())
        with loop_cm:
          for _it in range(iters):
            # ---- constant loads ---------------------------------------
            _half = CT * R // 2
            xtl_sb = const.tile([128, CT * R], dt)
            nc.sync.dma_start(out=xtl_sb[:, 0:_half], in_=xtl_d[:, 0:_half])
            nc.scalar.dma_start(out=xtl_sb[:, _half:], in_=xtl_d[:, _half:])
            ws1_sb = const.tile([128, CT * 8], dt)
            nc.sync.dma_start(out=ws1_sb, in_=ws1_d)
            identb = const.tile([128, 128], dt)
            nc.sync.dma_start(out=identb, in_=id_d)
            _hw = CT * HH // 2
            w1c_sb = const.tile([128, CT * HH], dt)
            nc.sync.dma_start(out=w1c_sb[:, 0:_hw], in_=w1c_d[:, 0:_hw])
            nc.scalar.dma_start(out=w1c_sb[:, _hw:], in_=w1c_d[:, _hw:])
            w2a_sb = const.tile([128, CT2 * F2p], dt)
            nc.scalar.dma_start(out=w2a_sb, in_=w2a_d)
            km_sb = const.tile([128, JT * R], dt)
            _hm = JT * R // 8
            for _q in range(8):
                nc.scalar.dma_start(
                    out=km_sb[:, _q * _hm: (_q + 1) * _hm],
                    in_=km_d[:, _q * _hm: (_q + 1) * _hm])

            if phases == "dma":
                zf = const.tile([128, F2], f32)
                nc.vector.memset(zf, 0.0)
                for rt in range(RT):
                    nc.sync.dma_start(out=out_d[rt * 128:(rt + 1) * 128, :],
                                      in_=zf)
                continue

            # ---- phase A: projections first (unblocks wb), then Wh ----
            snd = const.tile([128, RT * PW1], dt)
            wr_sb = const.tile([1, HEADS * R], dt)
            for rt in range(RT):
                pSt = psn.tile([128, HH], f32, tag=("s1" if rt % 2 else "s0"),
                               name=f"pSt{rt}")
                pS = pSt[:, 0:8]
                for ct in range(CT):
                    nc.tensor.matmul(
                        out=pS,
                        lhsT=xtl_sb[:, ct * R + rt * 128: ct * R + (rt + 1) * 128],
                        rhs=ws1_sb[:, ct * 8: (ct + 1) * 8],
                        start=(ct == 0), stop=(ct == CT - 1),
                    )
                b = rt * PW1
                # q = exp(s2), u = exp(alpha*s2); s2 = proj cols 4..8
                nc.scalar.activation(
                    out=snd[:, b + QOFF: b + QOFF + HEADS],
                    in_=pS[:, 4:8], func=Exp)
                nc.scalar.activation(
                    out=snd[:, b + UOFF: b + UOFF + HEADS],
                    in_=pS[:, 4:8], func=Exp, scale=ALPHA)
                # w = exp(-0.8*s1) computed from f32 psum in column form,
                # then transposed to a row per head (partition-0 reads only)
                wcol = work.tile([128, 4], dt, tag="wcol")
                nc.scalar.activation(out=wcol, in_=pS[:, 0:4], func=Exp,
                                     scale=ALPHA - 1.0)
                for h in range(HEADS):
                    pT = pstp.tile([128, 128], dt, tag="tp")
                    nc.tensor.transpose(out=pT[0:1, :], in_=wcol[:, h: h + 1],
                                        identity=identb)
                    nc.vector.tensor_copy(
                        out=wr_sb[0:1, h * R + rt * 128: h * R + (rt + 1) * 128],
                        in_=pT[0:1, :])

            # broadcast w rows per head (early: gates first zr)
            wb = const.tile([128, HEADS * R], dt)
            for h in range(HEADS):
                nc.gpsimd.partition_broadcast(
                    out_ap=wb[:, h * R: (h + 1) * R],
                    in_ap=wr_sb[0:1, h * R: (h + 1) * R])

            for rt in range(RT):
                pA = psn.tile([128, HH], f32, tag=f"n{rt}", name=f"pA{rt}")
                for ct in range(CT):
                    nc.tensor.matmul(
                        out=pA,
                        lhsT=xtl_sb[:, ct * R + rt * 128: ct * R + (rt + 1) * 128],
                        rhs=w1c_sb[:, ct * HH: (ct + 1) * HH],
                        start=(ct == 0), stop=(ct == CT - 1),
                    )
                b = rt * PW1
                for h in range(HEADS):
                    nc.scalar.copy(
                        out=snd[:, b + h * 129: b + h * 129 + 128],
                        in_=pA[:, h * 128: (h + 1) * 128])
                    nc.vector.memset(
                        snd[:, b + h * 129 + 128: b + h * 129 + 129], 1.0)
                nc.sync.dma_start(
                    out=whsend_t[:, b: b + PW1], in_=snd[:, b: b + PW1])

            # ---- gather Wh+q+u payload --------------------------------
            if collective:
                nc.gpsimd.collective_compute(
                    "AllGather", bypass,
                    replica_groups=[list(range(cfg.CORES))],
                    ins=[whsend_t.opt()], outs=[whfull_t.opt()],
                )
            wh_sb = const.tile([128, JT * PW1], dt)
            if collective:
                for c in range(cfg.CORES):
                    nc.sync.dma_start(
                        out=wh_sb[:, c * RT * PW1: (c + 1) * RT * PW1],
                        in_=whfull_t[c * 128: (c + 1) * 128, :])
            else:
                for c in range(cfg.CORES):
                    nc.sync.dma_start(
                        out=whfull_t[c * 128: (c + 1) * 128, :],
                        in_=whsend_t[:, :])
                    nc.sync.dma_start(
                        out=wh_sb[:, c * RT * PW1: (c + 1) * RT * PW1],
                        in_=whfull_t[c * 128: (c + 1) * 128, :])
            # f32 copies of the q/u scalar columns (tensor_scalar needs f32);
            # emitted lazily inside the first head's group loop so each only
            # waits on its own core block of the gather
            quf = const.tile([128, JT * 2 * HEADS], f32)
            wh_r = wh_sb[:, :].rearrange("p (t c) -> p t c", c=PW1)
            quf_r = quf[:, :].rearrange("p (t c) -> p t c", c=2 * HEADS)

            def quf_copy(c):
                nc.vector.tensor_copy(
                    out=quf_r[:, c * RT: (c + 1) * RT, :],
                    in_=wh_r[:, c * RT: (c + 1) * RT, QOFF: QOFF + 2 * HEADS])

            if phases == "wh":
                zf = const.tile([128, F2], f32)
                nc.vector.memset(zf, 0.0)
                for rt in range(RT):
                    nc.sync.dma_start(out=out_d[rt * 128:(rt + 1) * 128, :],
                                      in_=zf)
                continue

            # ---- layer 1 attention (group-major: consume gather blocks
            #      as they arrive; all 4 heads accumulate concurrently) ----
            hlocT = const.tile([128, CT2 * R], dt)
            dmp = (const.tile([128, 256], f32, name="dmp")
                   if dump in ("z00", "num0") else None)
            # (h, it) -> psum range: h<3 -> tag n{it} @ h*129;
            # h==3 -> spill tags s0 (it<3 @ it*129) / s1 (it==3 @ 0)
            psNt = [psn.tile([128, HH], f32, tag=f"n{it}", name=f"psNt{it}")
                    for it in range(RT)]
            psS0 = psn.tile([128, HH], f32, tag="s0", name="psS0")
            psS1 = psn.tile([128, HH], f32, tag="s1", name="psS1")

            def psn_range(h, it):
                if h < 3:
                    return psNt[it], h * 129
                if it < 3:
                    return psS0, it * 129
                return psS1, 0
            for g in range(NG):
                quf_copy(2 * g)
                quf_copy(2 * g + 1)
                for h in range(HEADS):
                    zr = work.tile([128, GRP * R], dt, tag="zr")
                    for k in range(GRP):
                        t = g * GRP + k
                        base = t * 2 * HEADS
                        nc.vector.tensor_scalar(
                            out=zr[:, k * R: (k + 1) * R],
                            in0=wb[:, h * R: (h + 1) * R],
                            scalar1=quf[:, base + HEADS + h: base + HEADS + h + 1],
                            scalar2=quf[:, base + h: base + h + 1],
                            op0=mult, op1=mx,
                        )
                    zg = wz.tile([128, GRP * R], dt, tag="zg")
                    _sp = pool_num * R
                    base_m = g * GRP * R
                    nc.vector.tensor_tensor(
                        out=zg[:, 0: GRP * R - _sp], in0=zr[:, 0: GRP * R - _sp],
                        in1=km_sb[:, base_m: base_m + GRP * R - _sp], op=mult)
                    if _sp:
                        nc.gpsimd.tensor_tensor(
                            out=zg[:, GRP * R - _sp:], in0=zr[:, GRP * R - _sp:],
                            in1=km_sb[:, base_m + GRP * R - _sp: base_m + GRP * R],
                            op=mult)
                    if dump == "z00" and g == 0 and h == 0:
                        nc.vector.tensor_copy(out=dmp, in_=zg[:, 0:256])
                    for k in range(GRP):
                        t = g * GRP + k
                        for it in range(RT):
                            pt_, off_ = psn_range(h, it)
                            # start=True resets the WHOLE psum bank: only the
                            # first chain per bank may use it (h==0 zeroes
                            # n{it}; h==3 it==0 zeroes s0, it==3 zeroes s1)
                            first = (t == 0 and (h == 0 or
                                     (h == 3 and it in (0, 3))))
                            nc.tensor.matmul(
                                out=pt_[:, off_: off_ + 129],
                                lhsT=zg[:, k * R + it * 128: k * R + (it + 1) * 128],
                                rhs=wh_sb[:, t * PW1 + h * 129: t * PW1 + (h + 1) * 129],
                                start=first, stop=(t == JT - 1),
                            )
            if dump == "num0":
                nc.vector.tensor_copy(out=dmp[:, 0:129], in_=psNt[0][:, 0:129])
                nc.vector.memset(dmp[:, 129:256], 0.0)
            # normalize + elu + transpose into hlocT (it-major), then the
            # layer-2 projection for that row tile accumulates immediately
            gsnd = const.tile([128, RT * PAY2], dt)
            w2r_sb = const.tile([1, R], dt)
            for it in range(RT):
                for h in range(HEADS):
                    pt_, off_ = psn_range(h, it)
                    rcp = work.tile([128, 1], f32, tag="rcp")
                    nc.vector.reciprocal(
                        out=rcp, in_=pt_[:, off_ + 128: off_ + 129])
                    hni = work.tile([128, 128], dt, tag="hni")
                    nc.scalar.activation(out=hni, in_=pt_[:, off_: off_ + 128],
                                         func=Copy, scale=rcp)
                    ehi = work.tile([128, 128], dt, tag="ehi")
                    nc.scalar.activation(out=ehi, in_=hni, func=Exp)
                    helu = work.tile([128, 128], dt, tag="helu")
                    nc.vector._custom_dve(
                        ELU_SEL, out=helu, in0=hni, in1=ehi,
                        s0=1.0, s1=0.0, imm2=0.0,
                    )
                    pT2 = pstp.tile([128, 128], dt, tag="tp")
                    nc.tensor.transpose(out=pT2, in_=helu, identity=identb)
                    nc.scalar.copy(
                        out=hlocT[:, h * R + it * 128: h * R + (it + 1) * 128],
                        in_=pT2)

            if phases == "l1":
                zf = const.tile([128, F2], f32)
                nc.vector.memset(zf, 0.0)
                for rt in range(RT):
                    nc.sync.dma_start(out=out_d[rt * 128:(rt + 1) * 128, :],
                                      in_=zf)
                continue

            if dump:
                df = const.tile([128, 256], f32)
                if dump in ("z00", "num0"):
                    nc.vector.tensor_copy(out=df, in_=dmp)
                elif dump == "hloc0":
                    nc.vector.tensor_copy(out=df, in_=hlocT[:, 0:256])
                elif dump == "wb0":
                    nc.vector.tensor_copy(out=df, in_=wb[:, 0:256])
                elif dump == "wh0":
                    nc.vector.tensor_copy(out=df, in_=wh_sb[:, 0:256])
                elif dump == "quf0":
                    nc.vector.tensor_copy(out=df, in_=quf[:, 0:256])
                nc.sync.dma_start(
                    out=out_d[:, :].rearrange("(a p) f -> p a f", p=128),
                    in_=df[:, :].rearrange("p (a f) -> p a f", f=F2))
                continue
            # ---- layer 2: local projections + gather payload ----------
            for rt in range(RT):
                pWt = psn.tile([128, HH], f32, tag=f"n{rt}", name=f"pWt{rt}")
                pW = pWt[:, 0:F2p]
                for ct2 in range(CT2):
                    nc.tensor.matmul(
                        out=pW,
                        lhsT=hlocT[:, ct2 * R + rt * 128: ct2 * R + (rt + 1) * 128],
                        rhs=w2a_sb[:, ct2 * F2p: (ct2 + 1) * F2p],
                        start=(ct2 == 0), stop=(ct2 == CT2 - 1),
                    )
                b2 = rt * PAY2
                nc.scalar.copy(out=gsnd[:, b2: b2 + F2], in_=pW[:, 0:F2])
                nc.vector.memset(gsnd[:, b2 + F2: b2 + F2 + 1], 1.0)
                nc.scalar.activation(
                    out=gsnd[:, b2 + F2 + 1: b2 + F2 + 2],
                    in_=pW[:, F2 + 1: F2 + 2], func=Exp)
                nc.scalar.activation(
                    out=gsnd[:, b2 + F2 + 2: b2 + F2 + 3],
                    in_=pW[:, F2 + 1: F2 + 2], func=Exp, scale=ALPHA)
                # w2 = exp(-0.8*s1_2) from f32 psum, then transpose
                w2col = work.tile([128, 1], dt, tag="w2col")
                nc.scalar.activation(out=w2col, in_=pW[:, F2: F2 + 1], func=Exp,
                                     scale=ALPHA - 1.0)
                pT3 = pstp.tile([128, 128], dt, tag="tp")
                nc.tensor.transpose(out=pT3[0:1, :], in_=w2col, identity=identb)
                nc.vector.tensor_copy(
                    out=w2r_sb[0:1, rt * 128: (rt + 1) * 128], in_=pT3[0:1, :])
            nc.sync.dma_start(out=gsend_t, in_=gsnd)

            w2b = const.tile([128, R], dt)
            nc.gpsimd.partition_broadcast(out_ap=w2b, in_ap=w2r_sb[0:1, :])

            if collective:
                nc.gpsimd.collective_compute(
                    "AllGather", bypass,
                    replica_groups=[list(range(cfg.CORES))],
                    ins=[gsend_t.opt()], outs=[gfull_t.opt()],
                )
            gf_sb = const.tile([128, JT * PAY2], dt)
            if not collective:
                for c in range(cfg.CORES):
                    _e = [nc.sync, nc.scalar][c % 2]
                    _e.dma_start(
                        out=gfull_t[c * 128: (c + 1) * 128, :],
                        in_=gsend_t[:, :])
            for cp in range(2):
                nc.sync.dma_start(
                    out=gf_sb[:, cp * 4 * RT * PAY2: (cp + 1) * 4 * RT * PAY2]
                        .rearrange("p (c w) -> p c w", c=4),
                    in_=gfull_t[cp * 512: (cp + 1) * 512, :]
                        .rearrange("(c p) w -> p c w", p=128))
            qu2f = const.tile([128, JT * 2], f32)
            gf_r = gf_sb[:, :].rearrange("p (t c) -> p t c", c=PAY2)
            qu2f_r = qu2f[:, :].rearrange("p (t c) -> p t c", c=2)

            def qu2f_copy(c):
                nc.vector.tensor_copy(
                    out=qu2f_r[:, c * RT: (c + 1) * RT, :],
                    in_=gf_r[:, c * RT: (c + 1) * RT, F2 + 1: F2 + 3])

            # ---- layer 2 attention ------------------------------------
            psOt = psn.tile([128, HH], f32, tag="s0", name="psOt")
            for g in range(NG):
                qu2f_copy(2 * g)
                qu2f_copy(2 * g + 1)
                zr = work.tile([128, GRP * R], dt, tag="zr")
                for k in range(GRP):
                    t = g * GRP + k
                    nc.vector.tensor_scalar(
                        out=zr[:, k * R: (k + 1) * R],
                        in0=w2b,
                        scalar1=qu2f[:, 2 * t + 1: 2 * t + 2],
                        scalar2=qu2f[:, 2 * t: 2 * t + 1],
                        op0=mult, op1=mx,
                    )
                zg = wz.tile([128, GRP * R], dt, tag="zg")
                _sp = pool_num * R
                base_m = g * GRP * R
                nc.vector.tensor_tensor(
                    out=zg[:, 0: GRP * R - _sp], in0=zr[:, 0: GRP * R - _sp],
                    in1=km_sb[:, base_m: base_m + GRP * R - _sp], op=mult)
                if _sp:
                    nc.gpsimd.tensor_tensor(
                        out=zg[:, GRP * R - _sp:], in0=zr[:, GRP * R - _sp:],
                        in1=km_sb[:, base_m + GRP * R - _sp: base_m + GRP * R],
                        op=mult)
                for k in range(GRP):
                    t = g * GRP + k
                    for it in range(RT):
                        nc.tensor.matmul(
                            out=psOt[:, it * 128: it * 128 + F2 + 1],
                            lhsT=zg[:, k * R + it * 128: k * R + (it + 1) * 128],
                            rhs=gf_sb[:, t * PAY2: t * PAY2 + F2 + 1],
                            start=(t == 0 and it == 0), stop=(t == JT - 1),
                        )

            # ---- finalize: normalize, store ---------------------------
            for it in range(RT):
                rc = work.tile([128, 1], f32, tag="rc")
                nc.vector.reciprocal(out=rc, in_=psOt[:, it * 128 + F2: it * 128 + F2 + 1])
                of = work.tile([128, F2], f32, tag="of")
                nc.scalar.activation(out=of, in_=psOt[:, it * 128: it * 128 + F2],
                                     func=Copy, scale=rc)
                nc.sync.dma_start(
                    out=out_d[it * 128: (it + 1) * 128, :], in_=of
                )

    nc.compile()
    return nc


# --------------------------------------------------------------------------
# Host-side prep / sharding
# --------------------------------------------------------------------------

def host_prep(cfg: Cfg, g, inputs, W1, a1, W2, a2):
    N, C, H, HEADS, F2, R = cfg.N, cfg.C, cfg.H, cfg.HEADS, cfg.F2, cfg.R
    X = np.asarray(inputs, np.float32)
    W1 = np.asarray(W1, np.float32)
    a1 = np.asarray(a1, np.float32)
    W2 = np.asarray(W2, np.float32)
    a2 = np.asarray(a2, np.float32)

    def tile128(A):
        # [k*128, cols] row-major -> partition-major [128, k*cols]
        k = A.shape[0] // 128
        return np.ascontiguousarray(
            A.reshape(k, 128, A.shape[1]).transpose(1, 0, 2).reshape(128, -1)
        )

    XT = np.ascontiguousarray(X.T).astype(BF16)                       # [C, N]
    w1c = tile128(np.ascontiguousarray(
        W1.transpose(1, 0, 2).reshape(C, HEADS * H)).astype(BF16))
    # fused score projections: cols 0..3 = W1[h] @ a1_first (s1),
    # cols 4..7 = W1[h] @ a1_second (s2)
    ws = np.zeros((C, 8), np.float32)
    for h in range(HEADS):
        ws[:, h] = W1[h] @ a1[h][:H, 0]
        ws[:, 4 + h] = W1[h] @ a1[h][H:, 0]
    ws1 = tile128(ws.astype(BF16))
    # layer-2 weights with fused a2 projection columns
    F2p = F2 + 2
    w2f = np.zeros((HEADS * H, F2p), np.float32)
    w2f[:, 0:F2] = W2
    w2f[:, F2] = W2 @ a2[:F2, 0]
    w2f[:, F2 + 1] = W2 @ a2[F2:, 0]
    w2a = tile128(w2f.astype(BF16))
    ident = np.eye(128, dtype=BF16)

    adj = np.asarray(g) > 0
    in_maps = []
    for c in range(cfg.CORES):
        rows = slice(c * R, (c + 1) * R)
        km = np.where(adj[rows].T, MASK_K, 0.0).astype(BF16)          # [N, R]
        in_maps.append({
            "xtloc": tile128(np.ascontiguousarray(XT[:, rows])),
            "km": tile128(km),
            "w1c": w1c, "ws1": ws1, "w2a": w2a,
            "ident": ident,
        })
    return in_maps


_NC_CACHE = {}


def get_compiled(cfg: Cfg):
    nc = _NC_CACHE.get(cfg)
    if nc is None:
        nc = build_gat_nc(cfg)
        _NC_CACHE[cfg] = nc
    return nc


def kernel(g, inputs, W1, a1, W2, a2):
    cfg = FULL
    nc = get_compiled(cfg)
    in_maps = host_prep(cfg, g, inputs, W1, a1, W2, a2)
    res = run_bass_kernel_spmd(nc, in_maps, core_ids=list(range(cfg.CORES)))
    out = np.concatenate(
        [np.asarray(res.results[c]["out"], np.float32) for c in range(cfg.CORES)],
        axis=0,
    )
    return out


# revision 43
# speedup vs baseline: 2.7203x; 2.1073x over previous
"""GAT (2-layer graph attention network) on 8 Trainium2 NeuronCores.

Strategy (1D node partition):
  - Each core owns R = N/8 rows (nodes) of the attention matrix.
  - Exp-free scores: softmax over column i is invariant to any per-i scale,
    so e^{lrelu(s1_i+s2_j)} / e^{s1_i} = max(q_j, w_i*u_j) with per-node
    q = e^{s2}, u = e^{0.2 s2}, w = e^{-0.8 s1}.  The N^2 path is then a
    single 4x-mode tensor_scalar (mult+max) plus one tensor_tensor min with
    a {0,K} mask tile (masking via min is exact: max(q,wu) > 0).  No
    activation-engine pass over N^2 elements at all.
  - Aggregation is "flipped": z-blocks [j,128i] are the PE stationary and
    [Wh_h | ones] streams 129 columns -> out[i, 129] accumulated over j in
    PSUM; column 128 is the softmax denominator for free (no separate
    ones-vector matmuls).
  - AllGather payload per node: 4x(Wh_h|1) + q,u per head (pre-exponentiated
    locally) = 524 cols; no replicated X compute, no full-N projections.
  - Layer 2 runs the same scheme with payload [Wh2|1|q2|u2].
"""

import math
from contextlib import ExitStack
from dataclasses import dataclass

import numpy as np
import ml_dtypes

import concourse.bass as bass
import concourse.mybir as mybir
import concourse.tile as tile
from concourse import bacc
from concourse.bass_utils import run_bass_kernel_spmd

BF16 = ml_dtypes.bfloat16
ALPHA = 0.2
MASK_K = 1.0

# --------------------------------------------------------------------------
# Custom fused DVE op (elu select), registered into concourse.dve_ops
# --------------------------------------------------------------------------

import concourse.dve_ops as dve_ops
from concourse.dve_spec import (
    Spec, Src0, Src1, C0, Zero, lower, select, _has_src1,
)
from concourse.dve_uop import DveOpSpec


def _make_elu_spec():
    # out = in0 > 0 ? in0 : in1 - s0   (elu with in1 = exp(in0), s0 = 1.0)
    def _elu_ref(in0, in1, s0, s1, imm2):
        x = in0.astype(np.float32)
        return np.where(x > 0, x, in1.astype(np.float32) - s0)

    return Spec(body=select(Src0 > Zero, Src0, Src1 - C0), reference=_elu_ref)


def _register(name, spec):
    if name in dve_ops._SUB_OPCODE_FOR_NAME:
        for op in dve_ops.OPS:
            if op.name == name:
                return op
    row = max(dve_ops._SUB_OPCODE_FOR_NAME.values()) + 1
    assert row < 0x20
    shas = {}
    for ver in ("v3", "v4"):
        uops = lower(spec, ver=ver)
        shas[ver] = DveOpSpec(
            name=name, opcode=row, uops=uops, rd1_en=_has_src1(spec)
        ).sha(ver)
    op = dve_ops.DveOp(name, spec, subdim=False, uops_sha=shas)
    dve_ops.OPS.append(op)
    dve_ops.CUSTOM_DVE_SPECS[name] = spec
    dve_ops._SUB_OPCODE_FOR_NAME[name] = row
    return op


ELU_SEL = _register("ELU_SEL_GAT", _make_elu_spec())


# --------------------------------------------------------------------------
# Kernel configuration
# --------------------------------------------------------------------------

@dataclass(frozen=True)
class Cfg:
    N: int = 4096      # nodes
    C: int = 512       # input feature dim
    H: int = 128       # hidden per head (must be 128)
    HEADS: int = 4
    F2: int = 64       # output dim
    CORES: int = 8
    GRP: int = 8       # j-tiles per batched mask-min

    @property
    def R(self): return self.N // self.CORES          # rows per core
    @property
    def JT(self): return self.N // 128                # j tiles
    @property
    def CT(self): return self.C // 128                # input-feature tiles
    @property
    def HH(self): return self.HEADS * self.H          # layer-1 out features
    @property
    def CT2(self): return self.HH // 128              # layer-2 contraction tiles
    @property
    def RT(self): return self.R // 128                # local row tiles
    @property
    def PW1(self): return self.HEADS * 129 + 2 * self.HEADS  # L1 payload cols
    @property
    def PAY2(self): return self.F2 + 3                # L2 payload cols
    @property
    def NG(self): return self.JT // self.GRP


FULL = Cfg()


# --------------------------------------------------------------------------
# Device program
# --------------------------------------------------------------------------

def build_gat_nc(cfg: Cfg, collective: bool = True, iters: int = 1,
                 loop_iters: int = 0, phases: str = "full",
                 pool_den: int = 8, pool_num: int = 3, dump: str = ""):
    dt = mybir.dt.bfloat16
    f32 = mybir.dt.float32
    add = mybir.AluOpType.add
    mult = mybir.AluOpType.mult
    mx = mybir.AluOpType.max
    mn = mybir.AluOpType.min
    bypass = mybir.AluOpType.bypass
    Exp = mybir.ActivationFunctionType.Exp
    Copy = mybir.ActivationFunctionType.Copy

    N, C, HEADS, F2, R = cfg.N, cfg.C, cfg.HEADS, cfg.F2, cfg.R
    JT, CT, HH, CT2, RT = cfg.JT, cfg.CT, cfg.HH, cfg.CT2, cfg.RT
    GRP, NG, PW1, PAY2 = cfg.GRP, cfg.NG, cfg.PW1, cfg.PAY2
    F2p = F2 + 2
    QOFF = HEADS * 129            # offset of q cols within a payload block
    UOFF = QOFF + HEADS           # offset of u cols

    nc = bacc.Bacc(
        "TRN2", target_bir_lowering=False, debug=False, num_devices=cfg.CORES
    )

    # ---- DRAM I/O -------------------------------------------------------
    xtl_d = nc.dram_tensor("xtloc", [128, CT * R], dt, kind="ExternalInput").ap()
    km_d = nc.dram_tensor("km", [128, JT * R], dt, kind="ExternalInput").ap()
    w1c_d = nc.dram_tensor("w1c", [128, CT * HH], dt, kind="ExternalInput").ap()
    ws1_d = nc.dram_tensor("ws1", [128, CT * 8], dt, kind="ExternalInput").ap()
    w2a_d = nc.dram_tensor("w2a", [128, CT2 * F2p], dt, kind="ExternalInput").ap()
    id_d = nc.dram_tensor("ident", [128, 128], dt, kind="ExternalInput").ap()
    out_d = nc.dram_tensor("out", [R, F2], f32, kind="ExternalOutput").ap()

    with tile.TileContext(nc) as tc, ExitStack() as ctx:
        const = ctx.enter_context(tc.tile_pool(name="const", bufs=1))
        work = ctx.enter_context(tc.tile_pool(name="work", bufs=3))
        wz = ctx.enter_context(tc.tile_pool(name="wz", bufs=3))
        psn = ctx.enter_context(tc.tile_pool(name="psn", bufs=1, space="PSUM"))
        pstp = ctx.enter_context(tc.tile_pool(name="pstp", bufs=2, space="PSUM"))
        dram = ctx.enter_context(tc.tile_pool(name="dram", bufs=1, space="DRAM"))

        whsend_t = dram.tile([128, RT * PW1], dt)
        gsend_t = dram.tile([128, RT * PAY2], dt)
        if cfg.CORES > 4:
            whfull_t = nc.dram_tensor(
                "whfull_sh", [cfg.CORES * 128, RT * PW1], dt,
                addr_space="Shared").ap()
            gfull_t = nc.dram_tensor(
                "gfull_sh", [cfg.CORES * 128, RT * PAY2], dt,
                addr_space="Shared").ap()
        else:
            whfull_t = dram.tile([cfg.CORES * 128, RT * PW1], dt)
            gfull_t = dram.tile([cfg.CORES * 128, RT * PAY2], dt)

        import contextlib
        loop_cm = (tc.For_i(0, loop_iters, 1) if loop_iters
                   else contextlib.nullcontext())
        with loop_cm:
          for _it in range(iters):
            # ---- constant loads ---------------------------------------
            _half = CT * R // 2
            xtl_sb = const.tile([128, CT * R], dt)
            nc.sync.dma_start(out=xtl_sb[:, 0:_half], in_=xtl_d[:, 0:_half])
            nc.scalar.dma_start(out=xtl_sb[:, _half:], in_=xtl_d[:, _half:])
            ws1_sb = const.tile([128, CT * 8], dt)
            nc.sync.dma_start(out=ws1_sb, in_=ws1_d)
            identb = const.tile([128, 128], dt)
            nc.sync.dma_start(out=identb, in_=id_d)
            _hw = CT * HH // 2
            w1c_sb = const.tile([128, CT * HH], dt)
            nc.sync.dma_start(out=w1c_sb[:, 0:_hw], in_=w1c_d[:, 0:_hw])
            nc.scalar.dma_start(out=w1c_sb[:, _hw:], in_=w1c_d[:, _hw:])
            w2a_sb = const.tile([128, CT2 * F2p], dt)
            nc.scalar.dma_start(out=w2a_sb, in_=w2a_d)
            km_sb = const.tile([128, JT * R], dt)
            _hm = JT * R // 8

            def km_load(q, eng):
                eng.dma_start(
                    out=km_sb[:, q * _hm: (q + 1) * _hm],
                    in_=km_d[:, q * _hm: (q + 1) * _hm])

            for _q in range(3):
                km_load(_q, nc.scalar)

            if phases == "dma":
                zf = const.tile([128, F2], f32)
                nc.vector.memset(zf, 0.0)
                for rt in range(RT):
                    nc.sync.dma_start(out=out_d[rt * 128:(rt + 1) * 128, :],
                                      in_=zf)
                continue

            # ---- phase A: projections first (unblocks wb), then Wh ----
            snd = const.tile([128, RT * PW1], dt)
            wr_sb = const.tile([1, HEADS * R], dt)
            for rt in range(RT):
                pSt = psn.tile([128, HH], f32, tag=("s1" if rt % 2 else "s0"),
                               name=f"pSt{rt}")
                pS = pSt[:, 0:8]
                for ct in range(CT):
                    nc.tensor.matmul(
                        out=pS,
                        lhsT=xtl_sb[:, ct * R + rt * 128: ct * R + (rt + 1) * 128],
                        rhs=ws1_sb[:, ct * 8: (ct + 1) * 8],
                        start=(ct == 0), stop=(ct == CT - 1),
                    )
                b = rt * PW1
                # q = exp(s2), u = exp(alpha*s2); s2 = proj cols 4..8
                nc.scalar.activation(
                    out=snd[:, b + QOFF: b + QOFF + HEADS],
                    in_=pS[:, 4:8], func=Exp)
                nc.scalar.activation(
                    out=snd[:, b + UOFF: b + UOFF + HEADS],
                    in_=pS[:, 4:8], func=Exp, scale=ALPHA)
                # w = exp(-0.8*s1) computed from f32 psum in column form,
                # then transposed to a row per head (partition-0 reads only)
                wcol = work.tile([128, 4], dt, tag="wcol")
                nc.scalar.activation(out=wcol, in_=pS[:, 0:4], func=Exp,
                                     scale=ALPHA - 1.0)
                for h in range(HEADS):
                    pT = pstp.tile([128, 128], dt, tag="tp")
                    nc.tensor.transpose(out=pT[0:1, :], in_=wcol[:, h: h + 1],
                                        identity=identb)
                    nc.vector.tensor_copy(
                        out=wr_sb[0:1, h * R + rt * 128: h * R + (rt + 1) * 128],
                        in_=pT[0:1, :])

            # broadcast w rows per head (early: gates first zr)
            wb = const.tile([128, HEADS * R], dt)
            for h in range(HEADS):
                nc.gpsimd.partition_broadcast(
                    out_ap=wb[:, h * R: (h + 1) * R],
                    in_ap=wr_sb[0:1, h * R: (h + 1) * R])

            for rt in range(RT):
                pA = psn.tile([128, HH], f32, tag=f"n{rt}", name=f"pA{rt}")
                for ct in range(CT):
                    nc.tensor.matmul(
                        out=pA,
                        lhsT=xtl_sb[:, ct * R + rt * 128: ct * R + (rt + 1) * 128],
                        rhs=w1c_sb[:, ct * HH: (ct + 1) * HH],
                        start=(ct == 0), stop=(ct == CT - 1),
                    )
                b = rt * PW1
                snd_wh = snd[:, b: b + HEADS * 129].rearrange(
                    "p (h c) -> p h c", c=129)[:, :, 0:128]
                pA_r = pA[:, :].rearrange("p (h c) -> p h c", c=128)
                if rt % 2 == 0:
                    nc.scalar.copy(out=snd_wh, in_=pA_r)
                else:
                    nc.vector.tensor_copy(out=snd_wh, in_=pA_r)
                nc.vector.memset(
                    snd[:, b: b + HEADS * 129].rearrange(
                        "p (h c) -> p h c", c=129)[:, :, 128:129], 1.0)
                nc.sync.dma_start(
                    out=whsend_t[:, b: b + PW1], in_=snd[:, b: b + PW1])

            # ---- gather Wh+q+u payload --------------------------------
            if collective:
                nc.gpsimd.collective_compute(
                    "AllGather", bypass,
                    replica_groups=[list(range(cfg.CORES))],
                    ins=[whsend_t.opt()], outs=[whfull_t.opt()],
                )
            wh_sb = const.tile([128, JT * PW1], dt)
            if collective:
                for c in range(cfg.CORES):
                    nc.sync.dma_start(
                        out=wh_sb[:, c * RT * PW1: (c + 1) * RT * PW1],
                        in_=whfull_t[c * 128: (c + 1) * 128, :])
                    if c < 5:
                        km_load(3 + c, nc.sync)
            else:
                for c in range(cfg.CORES):
                    nc.sync.dma_start(
                        out=whfull_t[c * 128: (c + 1) * 128, :],
                        in_=whsend_t[:, :])
                    nc.sync.dma_start(
                        out=wh_sb[:, c * RT * PW1: (c + 1) * RT * PW1],
                        in_=whfull_t[c * 128: (c + 1) * 128, :])
            # f32 copies of the q/u scalar columns (tensor_scalar needs f32);
            # emitted lazily inside the first head's group loop so each only
            # waits on its own core block of the gather
            quf = const.tile([128, JT * 2 * HEADS], f32)
            wh_r = wh_sb[:, :].rearrange("p (t c) -> p t c", c=PW1)
            quf_r = quf[:, :].rearrange("p (t c) -> p t c", c=2 * HEADS)

            def quf_copy(c):
                nc.vector.tensor_copy(
                    out=quf_r[:, c * RT: (c + 1) * RT, :],
                    in_=wh_r[:, c * RT: (c + 1) * RT, QOFF: QOFF + 2 * HEADS])

            if phases == "wh":
                zf = const.tile([128, F2], f32)
                nc.vector.memset(zf, 0.0)
                for rt in range(RT):
                    nc.sync.dma_start(out=out_d[rt * 128:(rt + 1) * 128, :],
                                      in_=zf)
                continue

            # ---- layer 1 attention (group-major: consume gather blocks
            #      as they arrive; all 4 heads accumulate concurrently) ----
            hlocT = const.tile([128, CT2 * R], dt)
            dmp = (const.tile([128, 256], f32, name="dmp")
                   if dump in ("z00", "num0") else None)
            # (h, it) -> psum range: h<3 -> tag n{it} @ h*129;
            # h==3 -> spill tags s0 (it<3 @ it*129) / s1 (it==3 @ 0)
            psNt = [psn.tile([128, HH], f32, tag=f"n{it}", name=f"psNt{it}")
                    for it in range(RT)]
            psS0 = psn.tile([128, HH], f32, tag="s0", name="psS0")
            psS1 = psn.tile([128, HH], f32, tag="s1", name="psS1")

            def psn_range(h, it):
                if h < 3:
                    return psNt[it], h * 129
                if it < 3:
                    return psS0, it * 129
                return psS1, 0
            for g in range(NG):
                quf_copy(2 * g)
                quf_copy(2 * g + 1)
                for h in range(HEADS):
                    zr = work.tile([128, GRP * R], dt, tag="zr")
                    for k in range(GRP):
                        t = g * GRP + k
                        base = t * 2 * HEADS
                        nc.vector.tensor_scalar(
                            out=zr[:, k * R: (k + 1) * R],
                            in0=wb[:, h * R: (h + 1) * R],
                            scalar1=quf[:, base + HEADS + h: base + HEADS + h + 1],
                            scalar2=quf[:, base + h: base + h + 1],
                            op0=mult, op1=mx,
                        )
                    zg = wz.tile([128, GRP * R], dt, tag="zg")
                    _sp = pool_num * R
                    base_m = g * GRP * R
                    nc.vector.tensor_tensor(
                        out=zg[:, 0: GRP * R - _sp], in0=zr[:, 0: GRP * R - _sp],
                        in1=km_sb[:, base_m: base_m + GRP * R - _sp], op=mult)
                    if _sp:
                        nc.gpsimd.tensor_tensor(
                            out=zg[:, GRP * R - _sp:], in0=zr[:, GRP * R - _sp:],
                            in1=km_sb[:, base_m + GRP * R - _sp: base_m + GRP * R],
                            op=mult)
                    if dump == "z00" and g == 0 and h == 0:
                        nc.vector.tensor_copy(out=dmp, in_=zg[:, 0:256])
                    for k in range(GRP):
                        t = g * GRP + k
                        for it in range(RT):
                            pt_, off_ = psn_range(h, it)
                            # start=True resets the WHOLE psum bank: only the
                            # first chain per bank may use it (h==0 zeroes
                            # n{it}; h==3 it==0 zeroes s0, it==3 zeroes s1)
                            first = (t == 0 and (h == 0 or
                                     (h == 3 and it in (0, 3))))
                            nc.tensor.matmul(
                                out=pt_[:, off_: off_ + 129],
                                lhsT=zg[:, k * R + it * 128: k * R + (it + 1) * 128],
                                rhs=wh_sb[:, t * PW1 + h * 129: t * PW1 + (h + 1) * 129],
                                start=first, stop=(t == JT - 1),
                            )
            if dump == "num0":
                nc.vector.tensor_copy(out=dmp[:, 0:129], in_=psNt[0][:, 0:129])
                nc.vector.memset(dmp[:, 129:256], 0.0)
            # normalize + elu + transpose into hlocT (it-major), then the
            # layer-2 projection for that row tile accumulates immediately
            gsnd = const.tile([128, RT * PAY2], dt)
            w2r_sb = const.tile([1, R], dt)
            for it in range(RT):
                for h in range(HEADS):
                    pt_, off_ = psn_range(h, it)
                    rcp = work.tile([128, 1], f32, tag="rcp")
                    nc.vector.reciprocal(
                        out=rcp, in_=pt_[:, off_ + 128: off_ + 129])
                    hni = work.tile([128, 128], dt, tag="hni")
                    nc.scalar.activation(out=hni, in_=pt_[:, off_: off_ + 128],
                                         func=Copy, scale=rcp)
                    ehi = work.tile([128, 128], dt, tag="ehi")
                    nc.scalar.activation(out=ehi, in_=hni, func=Exp)
                    helu = work.tile([128, 128], dt, tag="helu")
                    nc.vector._custom_dve(
                        ELU_SEL, out=helu, in0=hni, in1=ehi,
                        s0=1.0, s1=0.0, imm2=0.0,
                    )
                    pT2 = pstp.tile([128, 128], dt, tag="tp")
                    nc.tensor.transpose(out=pT2, in_=helu, identity=identb)
                    nc.scalar.copy(
                        out=hlocT[:, h * R + it * 128: h * R + (it + 1) * 128],
                        in_=pT2)

            if phases == "l1":
                zf = const.tile([128, F2], f32)
                nc.vector.memset(zf, 0.0)
                for rt in range(RT):
                    nc.sync.dma_start(out=out_d[rt * 128:(rt + 1) * 128, :],
                                      in_=zf)
                continue

            if dump:
                df = const.tile([128, 256], f32)
                if dump in ("z00", "num0"):
                    nc.vector.tensor_copy(out=df, in_=dmp)
                elif dump == "hloc0":
                    nc.vector.tensor_copy(out=df, in_=hlocT[:, 0:256])
                elif dump == "wb0":
                    nc.vector.tensor_copy(out=df, in_=wb[:, 0:256])
                elif dump == "wh0":
                    nc.vector.tensor_copy(out=df, in_=wh_sb[:, 0:256])
                elif dump == "quf0":
                    nc.vector.tensor_copy(out=df, in_=quf[:, 0:256])
                nc.sync.dma_start(
                    out=out_d[:, :].rearrange("(a p) f -> p a f", p=128),
                    in_=df[:, :].rearrange("p (a f) -> p a f", f=F2))
                continue
            # ---- layer 2: local projections + gather payload ----------
            for rt in range(RT):
                pWt = psn.tile([128, HH], f32, tag=f"n{rt}", name=f"pWt{rt}")
                pW = pWt[:, 0:F2p]
                for ct2 in range(CT2):
                    nc.tensor.matmul(
                        out=pW,
                        lhsT=hlocT[:, ct2 * R + rt * 128: ct2 * R + (rt + 1) * 128],
                        rhs=w2a_sb[:, ct2 * F2p: (ct2 + 1) * F2p],
                        start=(ct2 == 0), stop=(ct2 == CT2 - 1),
                    )
                b2 = rt * PAY2
                nc.scalar.copy(out=gsnd[:, b2: b2 + F2], in_=pW[:, 0:F2])
                nc.vector.memset(gsnd[:, b2 + F2: b2 + F2 + 1], 1.0)
                nc.scalar.activation(
                    out=gsnd[:, b2 + F2 + 1: b2 + F2 + 2],
                    in_=pW[:, F2 + 1: F2 + 2], func=Exp)
                nc.scalar.activation(
                    out=gsnd[:, b2 + F2 + 2: b2 + F2 + 3],
                    in_=pW[:, F2 + 1: F2 + 2], func=Exp, scale=ALPHA)
                # w2 = exp(-0.8*s1_2) from f32 psum, then transpose
                w2col = work.tile([128, 1], dt, tag="w2col")
                nc.scalar.activation(out=w2col, in_=pW[:, F2: F2 + 1], func=Exp,
                                     scale=ALPHA - 1.0)
                pT3 = pstp.tile([128, 128], dt, tag="tp")
                nc.tensor.transpose(out=pT3[0:1, :], in_=w2col, identity=identb)
                nc.vector.tensor_copy(
                    out=w2r_sb[0:1, rt * 128: (rt + 1) * 128], in_=pT3[0:1, :])
            nc.sync.dma_start(out=gsend_t, in_=gsnd)

            w2b = const.tile([128, R], dt)
            nc.gpsimd.partition_broadcast(out_ap=w2b, in_ap=w2r_sb[0:1, :])

            if collective:
                nc.gpsimd.collective_compute(
                    "AllGather", bypass,
                    replica_groups=[list(range(cfg.CORES))],
                    ins=[gsend_t.opt()], outs=[gfull_t.opt()],
                )
            gf_sb = const.tile([128, JT * PAY2], dt)
            if not collective:
                for c in range(cfg.CORES):
                    _e = [nc.sync, nc.scalar][c % 2]
                    _e.dma_start(
                        out=gfull_t[c * 128: (c + 1) * 128, :],
                        in_=gsend_t[:, :])
            for cp in range(2):
                nc.sync.dma_start(
                    out=gf_sb[:, cp * 4 * RT * PAY2: (cp + 1) * 4 * RT * PAY2]
                        .rearrange("p (c w) -> p c w", c=4),
                    in_=gfull_t[cp * 512: (cp + 1) * 512, :]
                        .rearrange("(c p) w -> p c w", p=128))
            qu2f = const.tile([128, JT * 2], f32)
            gf_r = gf_sb[:, :].rearrange("p (t c) -> p t c", c=PAY2)
            qu2f_r = qu2f[:, :].rearrange("p (t c) -> p t c", c=2)

            def qu2f_copy(c):
                nc.vector.tensor_copy(
                    out=qu2f_r[:, c * RT: (c + 1) * RT, :],
                    in_=gf_r[:, c * RT: (c + 1) * RT, F2 + 1: F2 + 3])

            # ---- layer 2 attention ------------------------------------
            psOt = psn.tile([128, HH], f32, tag="s0", name="psOt")
            for g in range(NG):
                qu2f_copy(2 * g)
                qu2f_copy(2 * g + 1)
                zr = work.tile([128, GRP * R], dt, tag="zr")
                for k in range(GRP):
                    t = g * GRP + k
                    nc.vector.tensor_scalar(
                        out=zr[:, k * R: (k + 1) * R],
                        in0=w2b,
                        scalar1=qu2f[:, 2 * t + 1: 2 * t + 2],
                        scalar2=qu2f[:, 2 * t: 2 * t + 1],
                        op0=mult, op1=mx,
                    )
                zg = wz.tile([128, GRP * R], dt, tag="zg")
                _sp = pool_num * R
                base_m = g * GRP * R
                nc.vector.tensor_tensor(
                    out=zg[:, 0: GRP * R - _sp], in0=zr[:, 0: GRP * R - _sp],
                    in1=km_sb[:, base_m: base_m + GRP * R - _sp], op=mult)
                if _sp:
                    nc.gpsimd.tensor_tensor(
                        out=zg[:, GRP * R - _sp:], in0=zr[:, GRP * R - _sp:],
                        in1=km_sb[:, base_m + GRP * R - _sp: base_m + GRP * R],
                        op=mult)
                for k in range(GRP):
                    t = g * GRP + k
                    for it in range(RT):
                        nc.tensor.matmul(
                            out=psOt[:, it * 128: it * 128 + F2 + 1],
                            lhsT=zg[:, k * R + it * 128: k * R + (it + 1) * 128],
                            rhs=gf_sb[:, t * PAY2: t * PAY2 + F2 + 1],
                            start=(t == 0 and it == 0), stop=(t == JT - 1),
                        )

            # ---- finalize: normalize, store ---------------------------
            for it in range(RT):
                rc = work.tile([128, 1], f32, tag="rc")
                nc.vector.reciprocal(out=rc, in_=psOt[:, it * 128 + F2: it * 128 + F2 + 1])
                of = work.tile([128, F2], f32, tag="of")
                nc.scalar.activation(out=of, in_=psOt[:, it * 128: it * 128 + F2],
                                     func=Copy, scale=rc)
                nc.sync.dma_start(
                    out=out_d[it * 128: (it + 1) * 128, :], in_=of
                )

    nc.compile()
    return nc


# --------------------------------------------------------------------------
# Host-side prep / sharding
# --------------------------------------------------------------------------

def host_prep(cfg: Cfg, g, inputs, W1, a1, W2, a2):
    N, C, H, HEADS, F2, R = cfg.N, cfg.C, cfg.H, cfg.HEADS, cfg.F2, cfg.R
    X = np.asarray(inputs, np.float32)
    W1 = np.asarray(W1, np.float32)
    a1 = np.asarray(a1, np.float32)
    W2 = np.asarray(W2, np.float32)
    a2 = np.asarray(a2, np.float32)

    def tile128(A):
        # [k*128, cols] row-major -> partition-major [128, k*cols]
        k = A.shape[0] // 128
        return np.ascontiguousarray(
            A.reshape(k, 128, A.shape[1]).transpose(1, 0, 2).reshape(128, -1)
        )

    XT = np.ascontiguousarray(X.T).astype(BF16)                       # [C, N]
    w1c = tile128(np.ascontiguousarray(
        W1.transpose(1, 0, 2).reshape(C, HEADS * H)).astype(BF16))
    # fused score projections: cols 0..3 = W1[h] @ a1_first (s1),
    # cols 4..7 = W1[h] @ a1_second (s2)
    ws = np.zeros((C, 8), np.float32)
    for h in range(HEADS):
        ws[:, h] = W1[h] @ a1[h][:H, 0]
        ws[:, 4 + h] = W1[h] @ a1[h][H:, 0]
    ws1 = tile128(ws.astype(BF16))
    # layer-2 weights with fused a2 projection columns
    F2p = F2 + 2
    w2f = np.zeros((HEADS * H, F2p), np.float32)
    w2f[:, 0:F2] = W2
    w2f[:, F2] = W2 @ a2[:F2, 0]
    w2f[:, F2 + 1] = W2 @ a2[F2:, 0]
    w2a = tile128(w2f.astype(BF16))
    ident = np.eye(128, dtype=BF16)

    adj = np.asarray(g) > 0
    in_maps = []
    for c in range(cfg.CORES):
        rows = slice(c * R, (c + 1) * R)
        km = np.where(adj[rows].T, MASK_K, 0.0).astype(BF16)          # [N, R]
        in_maps.append({
            "xtloc": tile128(np.ascontiguousarray(XT[:, rows])),
            "km": tile128(km),
            "w1c": w1c, "ws1": ws1, "w2a": w2a,
            "ident": ident,
        })
    return in_maps


_NC_CACHE = {}


def get_compiled(cfg: Cfg):
    nc = _NC_CACHE.get(cfg)
    if nc is None:
        nc = build_gat_nc(cfg)
        _NC_CACHE[cfg] = nc
    return nc


def kernel(g, inputs, W1, a1, W2, a2):
    cfg = FULL
    nc = get_compiled(cfg)
    in_maps = host_prep(cfg, g, inputs, W1, a1, W2, a2)
    res = run_bass_kernel_spmd(nc, in_maps, core_ids=list(range(cfg.CORES)))
    out = np.concatenate(
        [np.asarray(res.results[c]["out"], np.float32) for c in range(cfg.CORES)],
        axis=0,
    )
    return out
